# revision 12
# baseline (speedup 1.0000x reference)
"""Trainium2 Bass kernel for nn_ModelInverse.

Inverts a monotone scalar MLP F (PositiveLinear+Sigmoid stack, arch
[1,64,64,1], +1e-3*x monotonic term) at 2M targets z to well within the
reference bisection's 2e-2 relative-error gate.

g(z) = F^{-1}(z) is a smooth, nearly-linear scalar function fixed by the
(runtime) weights.  All weight-only work runs on the host in float64:
evaluate F on a dense grid, invert by monotone interpolation, and
least-squares-fit a degree-2 polynomial q(z) ~ g(z) at Chebyshev nodes
(max fit error ~7e-4, >20x inside the gate).  q is factored as
q(z) = (a*z + b)*(z + c) so the device evaluates it in exactly two fused
passes per element:

  ACT:  t = Identity(a*z + b)          (per-partition scale/bias)
  DVE:  y = (c + z) * t                (scalar_tensor_tensor)

Sharding: pure data parallel over the N axis across 8 cores; the three
coefficients are replicated; no cross-core comms.  Per core the kernel is
DMA-dominated: 1MB z in, 1MB y out, with chunked compute overlapped.
"""

import os
import sys

import numpy as np

for _p in ("/opt/trn_rl_repo", "/root/.axon_site/_ro/trn_rl_repo"):
    if os.path.isdir(_p) and _p not in sys.path:
        sys.path.insert(0, _p)

import concourse.bacc as bacc
import concourse.mybir as mybir
import concourse.tile as tile
from concourse.bass_utils import run_bass_kernel_spmd

F32 = mybir.dt.float32
F16 = mybir.dt.float16
AF = mybir.ActivationFunctionType
OP = mybir.AluOpType

N = 2_000_000
NCORES = 8
P = 128           # SBUF partitions
FREE = 1956       # elements per partition per core; 8*128*1956 = 2,002,944
SHARD = P * FREE  # 250,368 elements per core
CHUNKS = [440, 440, 440, 440, 196]   # graded: small tail chunk
assert sum(CHUNKS) == FREE
NCHUNK = len(CHUNKS)


def _build_program(a, b, c):
    nc = bacc.Bacc("TRN2", target_bir_lowering=False, debug=False,
                   num_devices=NCORES)

    # one flat [P, FCi] DRAM block per chunk
    z_d = [nc.dram_tensor(f"z{i}", [P, fc], F16, kind="ExternalInput")
           for i, fc in enumerate(CHUNKS)]
    o_d = [nc.dram_tensor(f"o{i}", [P, fc], F16, kind="ExternalOutput")
           for i, fc in enumerate(CHUNKS)]

    from contextlib import ExitStack
    with tile.TileContext(nc) as tc, ExitStack() as ctx:
        const = ctx.enter_context(tc.tile_pool(name="const", bufs=1))
        big = ctx.enter_context(tc.tile_pool(name="big", bufs=1))

        # dependency-free dummy activation: the framework inserts the ACT
        # table load right before it, hoisting the 1.3us load into the
        # preamble instead of the first real activation's wait chain
        dum = const.tile([1, 1], F32)
        nc.vector.memset(dum[:], 0.0)
        nc.scalar.activation(dum[:], dum[:], AF.Identity)
        # per-partition scale/bias registers for ACT, built by memset (no DMA)
        ca = const.tile([P, 1], F32)
        nc.vector.memset(ca[:], a)
        cb = const.tile([P, 1], F32)
        nc.vector.memset(cb[:], b)

        # all z chunks issue up front, spread across three queue owners so
        # no compute engine stalls on a DMA descriptor build mid-stream
        in_eng = [nc.sync, nc.sync, nc.scalar, nc.gpsimd, nc.gpsimd]
        out_eng = [nc.gpsimd, nc.gpsimd, nc.gpsimd, nc.sync, nc.sync]
        zts = []
        for i, fc in enumerate(CHUNKS):
            zt = big.tile([P, fc], F16, tag=f"zt{i}")
            in_eng[i].dma_start(zt[:], z_d[i].ap())
            zts.append(zt)

        for i, fc in enumerate(CHUNKS):
            zt = zts[i]
            t = big.tile([P, fc], F16, tag=f"t{i}")
            if i < NCHUNK - 1:
                nc.scalar.activation(t[:], zt[:], AF.Identity,
                                     bias=cb[:, 0:1], scale=ca[:, 0:1])
            else:
                # tail chunk's pass 1 on DVE (2x tensor_scalar) so it is
                # not queued behind the whole ACT stream
                nc.vector.tensor_scalar(t[:], zt[:], a, b,
                                        op0=OP.mult, op1=OP.add)
            y = big.tile([P, fc], F16, tag=f"y{i}")
            nc.vector.scalar_tensor_tensor(y[:], zt[:], c, t[:],
                                           op0=OP.add, op1=OP.mult)
            out_eng[i].dma_start(o_d[i].ap(), y[:])

    nc.compile()
    return nc


_NC_CACHE = {}


def _get_program(a, b, c):
    key = (a, b, c)
    if key not in _NC_CACHE:
        _NC_CACHE.clear()
        _NC_CACHE[key] = _build_program(a, b, c)
    return _NC_CACHE[key]


def _fit_coeffs(pre_w1, b1, pre_w2, b2, pre_w3, b3):
    """Host-side float64 fit of g = F^{-1} by a factored quadratic."""
    f64 = np.float64
    w1 = np.exp(np.asarray(pre_w1, f64))
    w2 = np.exp(np.asarray(pre_w2, f64))
    w3 = np.exp(np.asarray(pre_w3, f64))
    b1 = np.asarray(b1, f64).reshape(-1)
    b2 = np.asarray(b2, f64).reshape(-1)
    b3 = np.asarray(b3, f64).reshape(-1)

    def sig(v):
        return 1.0 / (1.0 + np.exp(-v))

    xs = np.linspace(0.0, 1.0, 32769)
    h = sig(xs[:, None] @ w1.T + b1)
    h = sig(h @ w2.T + b2)
    ax = (sig(h @ w3.T + b3).ravel() + 1e-3 * xs)
    Fs = (ax - ax[0]) / (ax[-1] - ax[0])

    # g at Chebyshev z-nodes via the monotone table; degree-2 LS fit in z
    Qn = 256
    zn = (np.cos((2 * np.arange(Qn) + 1) * np.pi / (2 * Qn)) + 1.0) / 2.0
    gn = np.interp(zn, Fs, xs)
    V = np.vander(zn, 3, increasing=True)
    q0, q1, q2 = np.linalg.lstsq(V, gn, rcond=None)[0]

    # q2 z^2 + q1 z + q0 == (a z + b)(z + c); c = small root (citardauq,
    # stable for q2 -> 0, where the form degrades to exactly linear)
    s = np.sqrt(max(q1 * q1 - 4.0 * q2 * q0, 0.0))
    den = q1 + s if q1 >= 0 else q1 - s
    c = 2.0 * q0 / den if den != 0 else 0.0
    a = q2
    b = q1 - q2 * c
    return float(a), float(b), float(c)


def _make_in_maps(z, pre_w1, b1, pre_w2, b2, pre_w3, b3):
    z = np.asarray(z).reshape(-1).astype(np.float16)
    assert z.size == N, z.shape
    zp = np.zeros(NCORES * SHARD, dtype=np.float16)
    zp[:N] = z
    shards = zp.reshape(NCORES, P, FREE)

    a, b, c = _fit_coeffs(pre_w1, b1, pre_w2, b2, pre_w3, b3)

    bounds = np.cumsum([0] + CHUNKS)
    in_maps = []
    for i in range(NCORES):
        m = {}
        for j in range(NCHUNK):
            m[f"z{j}"] = np.ascontiguousarray(
                shards[i, :, bounds[j]:bounds[j + 1]])
        in_maps.append(m)
    return (a, b, c), in_maps


def kernel(z, pre_w1, b1, pre_w2, b2, pre_w3, b3):
    (a, b, c), in_maps = _make_in_maps(z, pre_w1, b1, pre_w2, b2, pre_w3, b3)
    nc = _get_program(a, b, c)
    res = run_bass_kernel_spmd(nc, in_maps, list(range(NCORES))).results
    out = np.empty((NCORES, P, FREE), dtype=np.float32)
    bounds = np.cumsum([0] + CHUNKS)
    for i in range(NCORES):
        for j in range(NCHUNK):
            out[i, :, bounds[j]:bounds[j + 1]] = res[i][f"o{j}"]
    return out.reshape(-1)[:N].astype(np.float32).reshape(N, 1)


def profile_once(inputs):
    """Run once with tracing and return HW exec time in ns (test helper)."""
    (a, b, c), in_maps = _make_in_maps(**inputs)
    nc = _get_program(a, b, c)
    r = run_bass_kernel_spmd(nc, in_maps, list(range(NCORES)), trace=True)
    return r.exec_time_ns


# revision 14
# speedup vs baseline: 1.0270x; 1.0270x over previous
"""Trainium2 Bass kernel for nn_ModelInverse.

Inverts a monotone scalar MLP F (PositiveLinear+Sigmoid stack, arch
[1,64,64,1], +1e-3*x monotonic term) at 2M targets z to well within the
reference bisection's 2e-2 relative-error gate.

g(z) = F^{-1}(z) is a smooth, nearly-linear scalar function fixed by the
(runtime) weights.  All weight-only work runs on the host in float64:
evaluate F on a dense grid, invert by monotone interpolation, and
least-squares-fit a degree-2 polynomial q(z) ~ g(z) at Chebyshev nodes
(max fit error ~7e-4, >20x inside the gate).  q is factored as
q(z) = (a*z + b)*(z + c) so the device evaluates it in exactly two fused
passes per element:

  ACT:  t = Identity(a*z + b)          (per-partition scale/bias)
  DVE:  y = (c + z) * t                (scalar_tensor_tensor)

Sharding: pure data parallel over the N axis across 8 cores; the three
coefficients are replicated; no cross-core comms.  Per core the kernel is
DMA-dominated: 1MB z in, 1MB y out, with chunked compute overlapped.
"""

import os
import sys

import numpy as np

for _p in ("/opt/trn_rl_repo", "/root/.axon_site/_ro/trn_rl_repo"):
    if os.path.isdir(_p) and _p not in sys.path:
        sys.path.insert(0, _p)

import concourse.bacc as bacc
import concourse.mybir as mybir
import concourse.tile as tile
from concourse.bass_utils import run_bass_kernel_spmd

F32 = mybir.dt.float32
F16 = mybir.dt.float16
AF = mybir.ActivationFunctionType
OP = mybir.AluOpType

N = 2_000_000
NCORES = 8
P = 128           # SBUF partitions
FREE = 1956       # elements per partition per core; 8*128*1956 = 2,002,944
SHARD = P * FREE  # 250,368 elements per core
CHUNKS = [326, 543, 543, 544]   # graded: small head chunk starts compute early
assert sum(CHUNKS) == FREE
NCHUNK = len(CHUNKS)


def _build_program(a, b, c):
    nc = bacc.Bacc("TRN2", target_bir_lowering=False, debug=False,
                   num_devices=NCORES)

    # one flat [P, FCi] DRAM block per chunk
    z_d = [nc.dram_tensor(f"z{i}", [P, fc], F16, kind="ExternalInput")
           for i, fc in enumerate(CHUNKS)]
    o_d = [nc.dram_tensor(f"o{i}", [P, fc], F16, kind="ExternalOutput")
           for i, fc in enumerate(CHUNKS)]

    from contextlib import ExitStack
    with tile.TileContext(nc) as tc, ExitStack() as ctx:
        const = ctx.enter_context(tc.tile_pool(name="const", bufs=1))
        big = ctx.enter_context(tc.tile_pool(name="big", bufs=1))

        # dependency-free dummy activation: the framework inserts the ACT
        # table load right before it, hoisting the 1.3us load into the
        # preamble instead of the first real activation's wait chain
        dum = const.tile([1, 1], F32)
        nc.vector.memset(dum[:], 0.0)
        nc.scalar.activation(dum[:], dum[:], AF.Identity)
        # per-partition scale/bias registers for ACT, built by memset (no DMA)
        ca = const.tile([P, 1], F32)
        nc.vector.memset(ca[:], a)
        cb = const.tile([P, 1], F32)
        nc.vector.memset(cb[:], b)

        # z chunks issue up front, interleaved across the two HWDGE owners
        # so consecutive chunks transfer on different queues concurrently
        in_eng = [nc.sync, nc.scalar, nc.sync, nc.scalar]
        zts = []
        for i, fc in enumerate(CHUNKS):
            zt = big.tile([P, fc], F16, tag=f"zt{i}")
            in_eng[i].dma_start(zt[:], z_d[i].ap())
            zts.append(zt)

        ys = []
        for i, fc in enumerate(CHUNKS):
            zt = zts[i]
            t = big.tile([P, fc], F16, tag=f"t{i}")
            nc.scalar.activation(t[:], zt[:], AF.Identity,
                                 bias=cb[:, 0:1], scale=ca[:, 0:1])
            y = big.tile([P, fc], F16, tag=f"y{i}")
            nc.vector.scalar_tensor_tensor(y[:], zt[:], c, t[:],
                                           op0=OP.add, op1=OP.mult)
            ys.append(y)
            # early chunks stream out on the sync queue as soon as ready
            if i < 2:
                nc.sync.dma_start(o_d[i].ap(), y[:])
        # late chunks go out on the scalar queue, issued only after the
        # ACT stream is done so descriptor builds never stall an ACTIVATE
        for i in (2, 3):
            nc.scalar.dma_start(o_d[i].ap(), ys[i][:])

    nc.compile()
    return nc


_NC_CACHE = {}


def _get_program(a, b, c):
    key = (a, b, c)
    if key not in _NC_CACHE:
        _NC_CACHE.clear()
        _NC_CACHE[key] = _build_program(a, b, c)
    return _NC_CACHE[key]


def _fit_coeffs(pre_w1, b1, pre_w2, b2, pre_w3, b3):
    """Host-side float64 fit of g = F^{-1} by a factored quadratic."""
    f64 = np.float64
    w1 = np.exp(np.asarray(pre_w1, f64))
    w2 = np.exp(np.asarray(pre_w2, f64))
    w3 = np.exp(np.asarray(pre_w3, f64))
    b1 = np.asarray(b1, f64).reshape(-1)
    b2 = np.asarray(b2, f64).reshape(-1)
    b3 = np.asarray(b3, f64).reshape(-1)

    def sig(v):
        return 1.0 / (1.0 + np.exp(-v))

    xs = np.linspace(0.0, 1.0, 32769)
    h = sig(xs[:, None] @ w1.T + b1)
    h = sig(h @ w2.T + b2)
    ax = (sig(h @ w3.T + b3).ravel() + 1e-3 * xs)
    Fs = (ax - ax[0]) / (ax[-1] - ax[0])

    # g at Chebyshev z-nodes via the monotone table; degree-2 LS fit in z
    Qn = 256
    zn = (np.cos((2 * np.arange(Qn) + 1) * np.pi / (2 * Qn)) + 1.0) / 2.0
    gn = np.interp(zn, Fs, xs)
    V = np.vander(zn, 3, increasing=True)
    q0, q1, q2 = np.linalg.lstsq(V, gn, rcond=None)[0]

    # q2 z^2 + q1 z + q0 == (a z + b)(z + c); c = small root (citardauq,
    # stable for q2 -> 0, where the form degrades to exactly linear)
    s = np.sqrt(max(q1 * q1 - 4.0 * q2 * q0, 0.0))
    den = q1 + s if q1 >= 0 else q1 - s
    c = 2.0 * q0 / den if den != 0 else 0.0
    a = q2
    b = q1 - q2 * c
    return float(a), float(b), float(c)


def _make_in_maps(z, pre_w1, b1, pre_w2, b2, pre_w3, b3):
    z = np.asarray(z).reshape(-1).astype(np.float16)
    assert z.size == N, z.shape
    zp = np.zeros(NCORES * SHARD, dtype=np.float16)
    zp[:N] = z
    shards = zp.reshape(NCORES, P, FREE)

    a, b, c = _fit_coeffs(pre_w1, b1, pre_w2, b2, pre_w3, b3)

    bounds = np.cumsum([0] + CHUNKS)
    in_maps = []
    for i in range(NCORES):
        m = {}
        for j in range(NCHUNK):
            m[f"z{j}"] = np.ascontiguousarray(
                shards[i, :, bounds[j]:bounds[j + 1]])
        in_maps.append(m)
    return (a, b, c), in_maps


def kernel(z, pre_w1, b1, pre_w2, b2, pre_w3, b3):
    (a, b, c), in_maps = _make_in_maps(z, pre_w1, b1, pre_w2, b2, pre_w3, b3)
    nc = _get_program(a, b, c)
    res = run_bass_kernel_spmd(nc, in_maps, list(range(NCORES))).results
    out = np.empty((NCORES, P, FREE), dtype=np.float32)
    bounds = np.cumsum([0] + CHUNKS)
    for i in range(NCORES):
        for j in range(NCHUNK):
            out[i, :, bounds[j]:bounds[j + 1]] = res[i][f"o{j}"]
    return out.reshape(-1)[:N].astype(np.float32).reshape(N, 1)


def profile_once(inputs):
    """Run once with tracing and return HW exec time in ns (test helper)."""
    (a, b, c), in_maps = _make_in_maps(**inputs)
    nc = _get_program(a, b, c)
    r = run_bass_kernel_spmd(nc, in_maps, list(range(NCORES)), trace=True)
    return r.exec_time_ns
